# revision 1
# baseline (speedup 1.0000x reference)
"""AnchorStripeAttention Trainium2 kernel (8 NeuronCores, data-parallel).

Layout decisions (hardcoded for B=2, H=W=256, NH=6, DH=32, WS=8, AWS=4):
  - Shard over batch (2) x window-row-blocks (4): core c -> batch c//4,
    pixel rows [ (c%4)*64, (c%4+1)*64 ) -> 16384 contiguous tokens of qkv,
    32 contiguous rows of the 128-row anchor image. 256 windows per core.
  - CPB-MLP bias tables + logit scales are parameter-only -> precomputed on
    host, exp()'d and passed as small replicated constants.
  - Per 2-window chunk on device: load qk [128,384] + v [128,6,33] (ones
    column folded for softmax denominators), l2-normalize q/k (scales
    folded) token-major, transpose q/k/anchor to ch-major via TensorE,
    then per (window, head) a 4-matmul chain with zero transposes:
      MM1 attn1T[64,16] = k_sT.T @ anc_nT      (logits1, tokens on parts)
      e1T = exp(attn1T) * EB1                  (bias via exp-table mult)
      MM2 x1e[16,33]   = e1T.T @ [v|1]        (col 32 = softmax denom)
      x1n = x1e * recip(col32)                 (col 32 -> 1, reused below)
      MM3 attn2T[16,64] = anc_nT.T @ q_sT      (logits2, anchors on parts)
      e2T = exp(attn2T) * EB2T
      MM4 out[64,33]   = e2T.T @ x1n          (col 32 = denom2)
      out = out[:, :32] * recip(col32)
  Softmax max-subtraction is skipped: logits bounded by ~26, exp < 2e11.
  masks are all-zero per the problem spec and are not applied.
"""

import math
import numpy as np

B = 2
H = 256
WID = 256
NH = 6
DH = 32
CO = NH * DH
C = 3 * CO
WS = 8
AWS = 4
T = 121
NCORES = 8
NWR = 8         # window-rows per core
NWC = 32        # window-cols
TOK = 16384     # tokens per core shard

_NC_CACHE = {}


def _build_nc():
    import concourse.bass as bass
    from concourse import bacc
    import concourse.mybir as mybir
    from concourse.tile import TileContext
    from concourse.masks import make_identity

    f32 = mybir.dt.float32
    AX = mybir.AxisListType
    OP = mybir.AluOpType
    AF = mybir.ActivationFunctionType

    nc = bacc.Bacc("TRN2")
    qkv_d = nc.declare_dram_parameter("qkv", [NWR, WS, NWC, WS, C], f32, isOutput=False)
    anc_d = nc.declare_dram_parameter("anc", [NWR, AWS, NWC, AWS, CO], f32, isOutput=False)
    eb1_d = nc.declare_dram_parameter("eb1", [128, 96], f32, isOutput=False)
    eb2_d = nc.declare_dram_parameter("eb2t", [48, 384], f32, isOutput=False)
    scl_d = nc.declare_dram_parameter("scl", [128, 12], f32, isOutput=False)
    out_d = nc.declare_dram_parameter("out", [NWR, WS, NWC, WS, CO], f32, isOutput=True)

    with TileContext(nc) as tc:
        with (
            tc.tile_pool(name="const", bufs=1) as cpool,
            tc.tile_pool(name="io", bufs=3) as iopool,
            tc.tile_pool(name="work", bufs=2) as wpool,
            tc.tile_pool(name="small", bufs=3) as spool,
            tc.tile_pool(name="ps_tp", bufs=2, space="PSUM") as ps_tp,
            tc.tile_pool(name="ps_pa", bufs=1, space="PSUM") as ps_pa,
            tc.tile_pool(name="ps_o", bufs=1, space="PSUM") as ps_o,
        ):
            ident = cpool.tile([128, 128], f32)
            make_identity(nc, ident)
            eb1_t = cpool.tile([128, 96], f32)
            nc.sync.dma_start(out=eb1_t, in_=eb1_d[:, :])
            eb2_t = cpool.tile([48, 384], f32)
            nc.sync.dma_start(out=eb2_t, in_=eb2_d[:, :])
            scl_t = cpool.tile([128, 12], f32)
            nc.sync.dma_start(out=scl_t, in_=scl_d[:, :])

            for wr in range(NWR):
                anc_row = iopool.tile([16, NWC, 192], f32, tag="anc_row",
                                      name=f"anc_row_{wr}")
                for i in range(4):
                    nc.sync.dma_start(
                        out=anc_row[4 * i:4 * i + 4, :, :],
                        in_=anc_d[wr, i, :, :, :].rearrange("w j c -> j w c"))
                for wcp in range(NWC // 2):
                    c0 = 2 * wcp
                    # ---- loads: q,k,v for 2 windows in one tile ----
                    qk6 = iopool.tile([128, 576], f32, tag="qk6")
                    for w in range(2):
                        nc.sync.dma_start(
                            out=qk6[64 * w:64 * w + 64, :],
                            in_=qkv_d[wr, :, c0 + w, :, :])
                    v2 = wpool.tile([128, 6, 33], f32, tag="v2")
                    nc.gpsimd.memset(v2[:, :, 32:33], 1.0)
                    nc.gpsimd.tensor_copy(
                        out=v2[:, :, 0:32],
                        in_=qk6[:, 384:576].rearrange("p (h c) -> p h c", c=32))

                    # ---- l2 norms; q/k norms in ss[:,0:12], anchor norms
                    # (both windows) packed at ss[0:16, 12:24] ----
                    sq = wpool.tile([128, 384], f32, tag="sq")
                    nc.vector.tensor_mul(
                        out=sq, in0=qk6[:, 0:384], in1=qk6[:, 0:384])
                    ss = spool.tile([128, 24], f32, tag="ss")
                    nc.vector.tensor_reduce(
                        out=ss[:, 0:12],
                        in_=sq.rearrange("p (h c) -> p h c", c=32),
                        axis=AX.X, op=OP.add)
                    anc_p = anc_row[:, c0:c0 + 2, :]
                    asq = spool.tile([16, 2, 192], f32, tag="asq")
                    nc.vector.tensor_mul(out=asq, in0=anc_p, in1=anc_p)
                    nc.vector.tensor_reduce(
                        out=ss[0:16, 12:24],
                        in_=asq.rearrange("p w (h c) -> p (w h) c", c=32),
                        axis=AX.X, op=OP.add)
                    rss = spool.tile([128, 24], f32, tag="rss")
                    nc.vector.reciprocal(out=rss, in_=ss)
                    rs = spool.tile([128, 24], f32, tag="rs")
                    nc.scalar.sqrt(out=rs, in_=rss)
                    r = spool.tile([128, 12], f32, tag="r")
                    nc.vector.tensor_mul(out=r, in0=rs[:, 0:12], in1=scl_t)
                    qks = wpool.tile([128, 384], f32, tag="qks")
                    nc.vector.tensor_mul(
                        out=qks.rearrange("p (h c) -> p h c", c=32),
                        in0=qk6[:, 0:384].rearrange("p (h c) -> p h c", c=32),
                        in1=r[:, :, None].broadcast_to((128, 12, 32)))
                    anc_n = spool.tile([16, 2, 192], f32, tag="anc_n")
                    nc.vector.tensor_mul(
                        out=anc_n.rearrange("p w (h c) -> p w h c", c=32),
                        in0=anc_p.rearrange("p w (h c) -> p w h c", c=32),
                        in1=rs[0:16, 12:24].rearrange(
                            "p (w h) -> p w h", h=6)[:, :, :, None
                            ].broadcast_to((16, 2, 6, 32)))

                    # ---- ch-major transposes via TensorE ----
                    def tr(src_ap, kdim, mdim, tag):
                        ps = ps_tp.tile([mdim, 512], f32, tag="tp",
                                        name=f"tp_{tag}_{wr}_{wcp}")
                        nc.tensor.transpose(
                            out=ps[:, 0:kdim], in_=src_ap,
                            identity=ident[0:kdim, 0:kdim])
                        sb = wpool.tile([mdim, kdim], f32, tag=tag,
                                        name=f"{tag}_{wr}_{wcp}")
                        nc.vector.tensor_copy(out=sb, in_=ps[:, 0:kdim])
                        return sb

                    qT4 = tr(qks[:, 0:128], 128, 128, "qT4")
                    qT2 = tr(qks[:, 128:192], 128, 64, "qT2")
                    kT4 = tr(qks[:, 192:320], 128, 128, "kT4")
                    kT2 = tr(qks[:, 320:384], 128, 64, "kT2")
                    aT4 = [tr(anc_n[:, w, 0:128], 16, 128, f"aT4{w}")
                           for w in range(2)]
                    aT2 = [tr(anc_n[:, w, 128:192], 16, 64, f"aT2{w}")
                           for w in range(2)]

                    def head_slices(h, w):
                        if h < 4:
                            b = 32 * h
                            return (b, kT4[b:b + 32, 64 * w:64 * w + 64],
                                    qT4[b:b + 32, 64 * w:64 * w + 64],
                                    aT4[w][b:b + 32, :])
                        b = 32 * (h - 4)
                        return (b, kT2[b:b + 32, 64 * w:64 * w + 64],
                                qT2[b:b + 32, 64 * w:64 * w + 64],
                                aT2[w][b:b + 32, :])

                    # ---- stage 1 logits: concurrent row-group matmuls
                    # must land in different PSUM banks ----
                    pa = ps_pa.tile([128, 4, 512], f32, tag="pa")
                    for w in range(2):
                        for h in range(6):
                            b, kx, qx, ax = head_slices(h, w)
                            o_ap = (pa[64 * w:64 * w + 64, h, 0:16] if h < 4
                                    else pa[64 * w:64 * w + 64, h - 4, 16:32])
                            nc.tensor.matmul(
                                o_ap, lhsT=kx, rhs=ax, start=True, stop=True,
                                tile_position=(b, 64 * w))
                    e1x = wpool.tile([128, 6, 16], f32, tag="e1x")
                    nc.scalar.activation(
                        out=e1x[:, 0:4, :], in_=pa[:, 0:4, 0:16], func=AF.Exp)
                    nc.scalar.activation(
                        out=e1x[:, 4:6, :], in_=pa[:, 0:2, 16:32], func=AF.Exp)
                    e1 = wpool.tile([128, 6, 16], f32, tag="e1")
                    nc.vector.tensor_mul(
                        out=e1, in0=e1x,
                        in1=eb1_t.rearrange("p (h a) -> p h a", a=16))

                    # ---- stage 1 AV (denominator in col 32). Window w
                    # writes partitions 32w:32w+16 of pa bank 2w, columns
                    # 256:454 (bank choice keeps concurrently-running
                    # row-group matmuls in distinct banks). ----
                    for w in range(2):
                        for h in range(6):
                            nc.tensor.matmul(
                                pa[32 * w:32 * w + 16, 2 * w,
                                   256 + h * 33:256 + h * 33 + 33],
                                lhsT=e1[64 * w:64 * w + 64, h, :],
                                rhs=v2[64 * w:64 * w + 64, h, :],
                                start=True, stop=True,
                                tile_position=(64 * w, 32 * w))
                    rec1 = spool.tile([48, 6], f32, tag="rec1")
                    x1n = wpool.tile([48, 6, 33], f32, tag="x1n")
                    for w in range(2):
                        x1v = pa[32 * w:32 * w + 16, 2 * w, 256:454].rearrange(
                            "p (h c) -> p h c", c=33)
                        nc.vector.reciprocal(
                            out=rec1[32 * w:32 * w + 16, :], in_=x1v[:, :, 32])
                        nc.vector.tensor_mul(
                            out=x1n[32 * w:32 * w + 16], in0=x1v,
                            in1=rec1[32 * w:32 * w + 16, :, None
                                     ].broadcast_to((16, 6, 33)))

                    # ---- stage 2 logits ----
                    for w in range(2):
                        for h in range(6):
                            b, kx, qx, ax = head_slices(h, w)
                            o_ap = (pa[32 * w:32 * w + 16, h, 128:192]
                                    if h < 4 else
                                    pa[32 * w:32 * w + 16, h - 4, 192:256])
                            nc.tensor.matmul(
                                o_ap, lhsT=ax, rhs=qx, start=True, stop=True,
                                tile_position=(b, 32 * w))
                    e2x = wpool.tile([48, 6, 64], f32, tag="e2x")
                    nc.scalar.activation(
                        out=e2x[:, 0:4, :], in_=pa[0:48, 0:4, 128:192],
                        func=AF.Exp)
                    nc.scalar.activation(
                        out=e2x[:, 4:6, :], in_=pa[0:48, 0:2, 192:256],
                        func=AF.Exp)
                    e2 = wpool.tile([48, 6, 64], f32, tag="e2")
                    nc.vector.tensor_mul(
                        out=e2, in0=e2x,
                        in1=eb2_t.rearrange("p (h t) -> p h t", t=64))

                    # ---- stage 2 AV; window w -> its own oall bank ----
                    oall = ps_o.tile([128, 2, 512], f32, tag="oall")
                    for w in range(2):
                        for h in range(6):
                            nc.tensor.matmul(
                                oall[64 * w:64 * w + 64, w,
                                     h * 33:h * 33 + 33],
                                lhsT=e2[32 * w:32 * w + 16, h, :],
                                rhs=x1n[32 * w:32 * w + 16, h, :],
                                start=True, stop=True,
                                tile_position=(32 * w, 64 * w))

                    rec2 = spool.tile([128, 6], f32, tag="rec2")
                    osb = iopool.tile([128, 6, 32], f32, tag="osb")
                    for w in range(2):
                        ov = oall[64 * w:64 * w + 64, w, 0:198].rearrange(
                            "p (h c) -> p h c", c=33)
                        nc.vector.reciprocal(
                            out=rec2[64 * w:64 * w + 64, :], in_=ov[:, :, 32])
                        nc.vector.tensor_mul(
                            out=osb[64 * w:64 * w + 64], in0=ov[:, :, 0:32],
                            in1=rec2[64 * w:64 * w + 64, :, None
                                     ].broadcast_to((64, 6, 32)))
                    for w in range(2):
                        nc.sync.dma_start(
                            out=out_d[wr, :, c0 + w, :, :],
                            in_=osb[64 * w:64 * w + 64].rearrange(
                                "p h c -> p (h c)"))
    if not nc.is_finalized():
        nc.finalize()
    return nc


def _get_nc():
    if "nc" not in _NC_CACHE:
        _NC_CACHE["nc"] = _build_nc()
    return _NC_CACHE["nc"]


def _host_consts(table, i_a2w, i_w2a, ls1, ls2, w11, b11, w12, w21, b21, w22):
    def cpb_table(w1, b1, w2):
        hid = np.maximum(table.reshape(-1, 2) @ w1 + b1, 0.0)
        return hid @ w2  # (121, NH)

    def sigm(x):
        return 1.0 / (1.0 + np.exp(-x))

    bt1 = cpb_table(w11, b11, w12)
    bt2 = cpb_table(w21, b21, w22)
    # stage1 bias: (NH, 16, 64); stage2: (NH, 64, 16)
    b1 = 16.0 * sigm(bt1[i_a2w.reshape(-1)].reshape(16, 64, NH)).transpose(2, 0, 1)
    b2 = 16.0 * sigm(bt2[i_w2a.reshape(-1)].reshape(64, 16, NH)).transpose(2, 0, 1)
    # EB1[t, h, a] = exp(b1[h, a, t]); replicated for the 2-window partition dim
    eb1 = np.exp(b1).transpose(2, 0, 1).reshape(64, 96)
    eb1 = np.tile(eb1, (2, 1)).astype(np.float32)
    # EB2T[a, h, t] = exp(b2[h, t, a])
    eb2t = np.exp(b2).transpose(2, 0, 1).reshape(16, 384).astype(np.float32)
    eb2t = np.tile(eb2t, (3, 1))
    s1 = np.exp(np.minimum(ls1, math.log(100.0))).reshape(NH)
    s2 = np.exp(np.minimum(ls2, math.log(100.0))).reshape(NH)
    scl = np.tile(np.concatenate([s2, s1]).astype(np.float32), (128, 1))
    return eb1, eb2t, np.ascontiguousarray(scl)


def kernel(**inputs):
    kwargs = inputs
    from concourse.bass_utils import run_bass_kernel_spmd

    qkv = np.ascontiguousarray(np.asarray(inputs["qkv"], dtype=np.float32))
    anchor = np.ascontiguousarray(np.asarray(inputs["anchor"], dtype=np.float32))
    table = np.asarray(inputs["table"], dtype=np.float32)
    i_a2w = np.asarray(inputs["index_a2w"]).astype(np.int64)
    i_w2a = np.asarray(inputs["index_w2a"]).astype(np.int64)
    eb1, eb2t, scl = _host_consts(
        table, i_a2w, i_w2a,
        np.asarray(inputs["logit_scale1"], np.float32),
        np.asarray(inputs["logit_scale2"], np.float32),
        np.asarray(inputs["cpb1_w1"], np.float32),
        np.asarray(inputs["cpb1_b1"], np.float32),
        np.asarray(inputs["cpb1_w2"], np.float32),
        np.asarray(inputs["cpb2_w1"], np.float32),
        np.asarray(inputs["cpb2_b1"], np.float32),
        np.asarray(inputs["cpb2_w2"], np.float32),
    )

    in_maps = []
    for c in range(NCORES):
        b = c // 4
        rb = c % 4
        qkv_sh = qkv[b, rb * TOK:(rb + 1) * TOK].reshape(NWR, WS, NWC, WS, C)
        anc_sh = anchor[b, rb * 32:(rb + 1) * 32].reshape(NWR, AWS, NWC, AWS, CO)
        in_maps.append({
            "qkv": np.ascontiguousarray(qkv_sh),
            "anc": np.ascontiguousarray(anc_sh),
            "eb1": eb1, "eb2t": eb2t, "scl": scl,
        })

    nc = _get_nc()
    trace = bool(kwargs.get("_trace"))
    tkw = {}
    if trace:
        tkw = dict(trace=True, tmpdir=kwargs.get("_tmpdir"))
    res = run_bass_kernel_spmd(nc, in_maps, list(range(NCORES)), **tkw)
    results = res.results if hasattr(res, "results") else res
    if trace:
        kernel._last_profile = res

    out = np.empty((B, H * WID, CO), dtype=np.float32)
    for c in range(NCORES):
        b = c // 4
        rb = c % 4
        out[b, rb * TOK:(rb + 1) * TOK] = np.asarray(
            results[c]["out"], dtype=np.float32).reshape(TOK, CO)
    return out



# revision 5
# speedup vs baseline: 1.3992x; 1.3992x over previous
"""AnchorStripeAttention Trainium2 kernel (8 NeuronCores, data-parallel).

Layout decisions (hardcoded for B=2, H=W=256, NH=6, DH=32, WS=8, AWS=4):
  - Shard over batch (2) x window-row-blocks (4): core c -> batch c//4,
    pixel rows [ (c%4)*64, (c%4+1)*64 ) -> 16384 contiguous tokens of qkv,
    32 contiguous rows of the 128-row anchor image. 256 windows per core.
  - CPB-MLP bias tables + logit scales are parameter-only -> precomputed on
    host, exp()'d and passed as small replicated bf16 constants.
  - All matmul operands bf16 (1 cyc/row vs fp32's 2x half-speed pumps);
    HBM traffic halved; output returned bf16 and upcast on host.
  - Per window-row, chunks of 2 windows are processed in groups of 8 in two
    phases: (1) loads + l2-norm factors (scalar.sqrt batched), (2) scaling,
    TensorE transposes, the 4-matmul attention chain (scalar Exp batched).
    This keeps Sqrt/Exp act-table swaps to 2 per group instead of 2 per
    chunk (tables live in different act-table sets -> each swap is 1.3us).
  - Per (window, head) 4-matmul chain with zero transposes:
      MM1 attn1T[64,16] = k_sT.T @ anc_nT      (logits1, tokens on parts)
      e1T = exp(attn1T) * EB1                  (bias via exp-table mult)
      MM2 x1e[16,33]   = e1T.T @ [v|1]        (col 32 = softmax denom)
      x1n = x1e * recip(col32)                 (col 32 -> 1, reused below)
      MM3 attn2T[16,64] = anc_nT.T @ q_sT      (logits2, anchors on parts)
      e2T = exp(attn2T) * EB2T
      MM4 out[64,33]   = e2T.T @ x1n          (col 32 = denom2)
      out = out[:, :32] * recip(col32)
  Softmax max-subtraction is skipped: logits bounded by ~26, exp < 2e11.
  masks are all-zero per the problem spec and are not applied.
"""

import math
import numpy as np

B = 2
H = 256
WID = 256
NH = 6
DH = 32
CO = NH * DH
C = 3 * CO
WS = 8
AWS = 4
T = 121
NCORES = 8
NWR = 8         # window-rows per core
NWC = 32        # window-cols
TOK = 16384     # tokens per core shard
GRP = 8         # chunks per norm/attention phase group

_NC_CACHE = {}


def _build_nc():
    import concourse.bass as bass
    from concourse import bacc
    import concourse.mybir as mybir
    from concourse.tile import TileContext
    from concourse.masks import make_identity

    f32 = mybir.dt.float32
    bf16 = mybir.dt.bfloat16
    AX = mybir.AxisListType
    OP = mybir.AluOpType
    AF = mybir.ActivationFunctionType

    nc = bacc.Bacc("TRN2")
    qkv_d = nc.declare_dram_parameter("qkv", [NWR, WS, NWC, WS, C], bf16, isOutput=False)
    anc_d = nc.declare_dram_parameter("anc", [NWR, AWS, NWC, AWS, CO], bf16, isOutput=False)
    eb1_d = nc.declare_dram_parameter("eb1", [128, 96], bf16, isOutput=False)
    eb2_d = nc.declare_dram_parameter("eb2t", [48, 384], bf16, isOutput=False)
    scl_d = nc.declare_dram_parameter("scl", [128, 12], bf16, isOutput=False)
    out_d = nc.declare_dram_parameter("out", [NWR, WS, NWC, WS, CO], bf16, isOutput=True)

    with TileContext(nc) as tc:
        with (
            tc.tile_pool(name="const", bufs=1) as cpool,
            tc.tile_pool(name="qk", bufs=2) as qkpool,
            tc.tile_pool(name="io", bufs=3) as iopool,
            tc.tile_pool(name="work", bufs=2) as wpool,
            tc.tile_pool(name="small", bufs=3) as spool,
            tc.tile_pool(name="norm", bufs=2) as npool,
            tc.tile_pool(name="ps_tp", bufs=2, space="PSUM") as ps_tp,
            tc.tile_pool(name="ps_pa", bufs=1, space="PSUM") as ps_pa,
            tc.tile_pool(name="ps_o", bufs=1, space="PSUM") as ps_o,
        ):
            ident = cpool.tile([128, 128], bf16)
            make_identity(nc, ident)
            eb1_t = cpool.tile([128, 96], bf16)
            nc.sync.dma_start(out=eb1_t, in_=eb1_d[:, :])
            eb2_t = cpool.tile([48, 384], bf16)
            nc.sync.dma_start(out=eb2_t, in_=eb2_d[:, :])
            scl_t = cpool.tile([128, 12], bf16)
            nc.sync.dma_start(out=scl_t, in_=scl_d[:, :])

            for wr in range(NWR):
                anc_row = iopool.tile([16, NWC, 192], bf16, tag="anc_row",
                                      name=f"anc_row_{wr}")
                for i in range(4):
                    nc.sync.dma_start(
                        out=anc_row[4 * i:4 * i + 4, :, :],
                        in_=anc_d[wr, i, :, :, :].rearrange("w j c -> j w c"))
                for half in range(2):
                    qk_t = {}
                    rs_t = {}
                    r_t = {}
                    # ---- phase 1: loads + l2-norm factors (sqrt batched) ----
                    for j in range(GRP):
                        wcp = half * GRP + j
                        c0 = 2 * wcp
                        qk6 = qkpool.tile([128, 576], bf16, tag=f"qk6_{j}",
                                          name=f"qk6_{wr}_{wcp}")
                        for w in range(2):
                            nc.sync.dma_start(
                                out=qk6[64 * w:64 * w + 64, :],
                                in_=qkv_d[wr, :, c0 + w, :, :])
                        sq = wpool.tile([128, 384], bf16, tag="sq")
                        nc.vector.tensor_mul(
                            out=sq, in0=qk6[:, 0:384], in1=qk6[:, 0:384])
                        ss = spool.tile([128, 24], f32, tag="ss")
                        nc.vector.tensor_reduce(
                            out=ss[:, 0:12],
                            in_=sq.rearrange("p (h c) -> p h c", c=32),
                            axis=AX.X, op=OP.add)
                        anc_p = anc_row[:, c0:c0 + 2, :]
                        asq = spool.tile([16, 2, 192], bf16, tag="asq")
                        nc.vector.tensor_mul(out=asq, in0=anc_p, in1=anc_p)
                        nc.vector.tensor_reduce(
                            out=ss[0:16, 12:24],
                            in_=asq.rearrange("p w (h c) -> p (w h) c", c=32),
                            axis=AX.X, op=OP.add)
                        rss = spool.tile([128, 24], f32, tag="rss")
                        nc.vector.reciprocal(out=rss, in_=ss)
                        rs = npool.tile([128, 24], bf16, tag=f"rs_{j}",
                                        name=f"rs_{wr}_{wcp}")
                        nc.scalar.sqrt(out=rs, in_=rss)
                        r = npool.tile([128, 12], bf16, tag=f"r_{j}",
                                       name=f"r_{wr}_{wcp}")
                        nc.vector.tensor_mul(out=r, in0=rs[:, 0:12], in1=scl_t)
                        qk_t[j] = qk6
                        rs_t[j] = rs
                        r_t[j] = r

                    # ---- phase 2: scale, transpose, attention (exp batched) ----
                    for j in range(GRP):
                        wcp = half * GRP + j
                        c0 = 2 * wcp
                        qk6 = qk_t[j]
                        rs = rs_t[j]
                        r = r_t[j]
                        anc_p = anc_row[:, c0:c0 + 2, :]

                        v2 = wpool.tile([128, 6, 33], bf16, tag="v2")
                        nc.gpsimd.memset(v2[:, :, 32:33], 1.0)
                        nc.gpsimd.tensor_copy(
                            out=v2[:, :, 0:32],
                            in_=qk6[:, 384:576].rearrange("p (h c) -> p h c", c=32))

                        qks = wpool.tile([128, 384], bf16, tag="qks")
                        nc.vector.tensor_mul(
                            out=qks.rearrange("p (h c) -> p h c", c=32),
                            in0=qk6[:, 0:384].rearrange("p (h c) -> p h c", c=32),
                            in1=r[:, :, None].broadcast_to((128, 12, 32)))
                        anc_n = spool.tile([16, 2, 192], bf16, tag="anc_n")
                        nc.vector.tensor_mul(
                            out=anc_n.rearrange("p w (h c) -> p w h c", c=32),
                            in0=anc_p.rearrange("p w (h c) -> p w h c", c=32),
                            in1=rs[0:16, 12:24].rearrange(
                                "p (w h) -> p w h", h=6)[:, :, :, None
                                ].broadcast_to((16, 2, 6, 32)))

                        # ---- ch-major transposes via TensorE ----
                        def tr(src_ap, kdim, mdim, tag):
                            ps = ps_tp.tile([mdim, 1024], bf16, tag="tp",
                                            name=f"tp_{tag}_{wr}_{wcp}")
                            nc.tensor.transpose(
                                out=ps[:, 0:kdim], in_=src_ap,
                                identity=ident[0:kdim, 0:kdim])
                            sb = wpool.tile([mdim, kdim], bf16, tag=tag,
                                            name=f"{tag}_{wr}_{wcp}")
                            nc.vector.tensor_copy(out=sb, in_=ps[:, 0:kdim])
                            return sb

                        qT4 = tr(qks[:, 0:128], 128, 128, "qT4")
                        qT2 = tr(qks[:, 128:192], 128, 64, "qT2")
                        kT4 = tr(qks[:, 192:320], 128, 128, "kT4")
                        kT2 = tr(qks[:, 320:384], 128, 64, "kT2")
                        aT4 = [tr(anc_n[:, w, 0:128], 16, 128, f"aT4{w}")
                               for w in range(2)]
                        aT2 = [tr(anc_n[:, w, 128:192], 16, 64, f"aT2{w}")
                               for w in range(2)]

                        def head_slices(h, w):
                            if h < 4:
                                b = 32 * h
                                return (b, kT4[b:b + 32, 64 * w:64 * w + 64],
                                        qT4[b:b + 32, 64 * w:64 * w + 64],
                                        aT4[w][b:b + 32, :])
                            b = 32 * (h - 4)
                            return (b, kT2[b:b + 32, 64 * w:64 * w + 64],
                                    qT2[b:b + 32, 64 * w:64 * w + 64],
                                    aT2[w][b:b + 32, :])

                        # ---- stage 1 logits: concurrent row-group matmuls
                        # must land in different PSUM banks ----
                        pa = ps_pa.tile([128, 4, 512], f32, tag="pa")
                        for w in range(2):
                            for h in range(6):
                                b, kx, qx, ax = head_slices(h, w)
                                o_ap = (pa[64 * w:64 * w + 64, h, 0:16] if h < 4
                                        else pa[64 * w:64 * w + 64, h - 4, 16:32])
                                nc.tensor.matmul(
                                    o_ap, lhsT=kx, rhs=ax, start=True, stop=True,
                                    tile_position=(b, 64 * w))
                        e1x = wpool.tile([128, 6, 16], bf16, tag="e1x")
                        nc.scalar.activation(
                            out=e1x[:, 0:4, :], in_=pa[:, 0:4, 0:16], func=AF.Exp)
                        nc.scalar.activation(
                            out=e1x[:, 4:6, :], in_=pa[:, 0:2, 16:32], func=AF.Exp)
                        e1 = wpool.tile([128, 6, 16], bf16, tag="e1")
                        nc.vector.tensor_mul(
                            out=e1, in0=e1x,
                            in1=eb1_t.rearrange("p (h a) -> p h a", a=16))

                        # ---- stage 1 AV (denominator in col 32) ----
                        for w in range(2):
                            for h in range(6):
                                nc.tensor.matmul(
                                    pa[32 * w:32 * w + 16, 2 * w,
                                       256 + h * 33:256 + h * 33 + 33],
                                    lhsT=e1[64 * w:64 * w + 64, h, :],
                                    rhs=v2[64 * w:64 * w + 64, h, :],
                                    start=True, stop=True,
                                    tile_position=(64 * w, 32 * w))
                        rec1 = spool.tile([48, 6], f32, tag="rec1")
                        x1n = wpool.tile([48, 6, 33], bf16, tag="x1n")
                        for w in range(2):
                            x1v = pa[32 * w:32 * w + 16, 2 * w, 256:454].rearrange(
                                "p (h c) -> p h c", c=33)
                            nc.vector.reciprocal(
                                out=rec1[32 * w:32 * w + 16, :], in_=x1v[:, :, 32])
                            nc.vector.tensor_mul(
                                out=x1n[32 * w:32 * w + 16], in0=x1v,
                                in1=rec1[32 * w:32 * w + 16, :, None
                                         ].broadcast_to((16, 6, 33)))

                        # ---- stage 2 logits ----
                        for w in range(2):
                            for h in range(6):
                                b, kx, qx, ax = head_slices(h, w)
                                o_ap = (pa[32 * w:32 * w + 16, h, 128:192]
                                        if h < 4 else
                                        pa[32 * w:32 * w + 16, h - 4, 192:256])
                                nc.tensor.matmul(
                                    o_ap, lhsT=ax, rhs=qx, start=True, stop=True,
                                    tile_position=(b, 32 * w))
                        e2x = wpool.tile([48, 6, 64], bf16, tag="e2x")
                        nc.scalar.activation(
                            out=e2x[:, 0:4, :], in_=pa[0:48, 0:4, 128:192],
                            func=AF.Exp)
                        nc.scalar.activation(
                            out=e2x[:, 4:6, :], in_=pa[0:48, 0:2, 192:256],
                            func=AF.Exp)
                        e2 = wpool.tile([48, 6, 64], bf16, tag="e2")
                        nc.vector.tensor_mul(
                            out=e2, in0=e2x,
                            in1=eb2_t.rearrange("p (h t) -> p h t", t=64))

                        # ---- stage 2 AV; window w -> its own oall bank ----
                        oall = ps_o.tile([128, 2, 512], f32, tag="oall")
                        for w in range(2):
                            for h in range(6):
                                nc.tensor.matmul(
                                    oall[64 * w:64 * w + 64, w,
                                         h * 33:h * 33 + 33],
                                    lhsT=e2[32 * w:32 * w + 16, h, :],
                                    rhs=x1n[32 * w:32 * w + 16, h, :],
                                    start=True, stop=True,
                                    tile_position=(32 * w, 64 * w))

                        rec2 = spool.tile([128, 6], f32, tag="rec2")
                        osb = iopool.tile([128, 6, 32], bf16, tag="osb")
                        for w in range(2):
                            ov = oall[64 * w:64 * w + 64, w, 0:198].rearrange(
                                "p (h c) -> p h c", c=33)
                            nc.vector.reciprocal(
                                out=rec2[64 * w:64 * w + 64, :], in_=ov[:, :, 32])
                            nc.vector.tensor_mul(
                                out=osb[64 * w:64 * w + 64], in0=ov[:, :, 0:32],
                                in1=rec2[64 * w:64 * w + 64, :, None
                                         ].broadcast_to((64, 6, 32)))
                        for w in range(2):
                            nc.sync.dma_start(
                                out=out_d[wr, :, c0 + w, :, :],
                                in_=osb[64 * w:64 * w + 64].rearrange(
                                    "p h c -> p (h c)"))
    if not nc.is_finalized():
        nc.finalize()
    return nc


def _get_nc():
    if "nc" not in _NC_CACHE:
        _NC_CACHE["nc"] = _build_nc()
    return _NC_CACHE["nc"]


def _host_consts(table, i_a2w, i_w2a, ls1, ls2, w11, b11, w12, w21, b21, w22):
    def cpb_table(w1, b1, w2):
        hid = np.maximum(table.reshape(-1, 2) @ w1 + b1, 0.0)
        return hid @ w2  # (121, NH)

    def sigm(x):
        return 1.0 / (1.0 + np.exp(-x))

    bt1 = cpb_table(w11, b11, w12)
    bt2 = cpb_table(w21, b21, w22)
    # stage1 bias: (NH, 16, 64); stage2: (NH, 64, 16)
    b1 = 16.0 * sigm(bt1[i_a2w.reshape(-1)].reshape(16, 64, NH)).transpose(2, 0, 1)
    b2 = 16.0 * sigm(bt2[i_w2a.reshape(-1)].reshape(64, 16, NH)).transpose(2, 0, 1)
    # EB1[t, h, a] = exp(b1[h, a, t]); replicated for the 2-window partition dim
    eb1 = np.exp(b1).transpose(2, 0, 1).reshape(64, 96)
    eb1 = np.tile(eb1, (2, 1)).astype(np.float32)
    # EB2T[a, h, t] = exp(b2[h, t, a])
    eb2t = np.exp(b2).transpose(2, 0, 1).reshape(16, 384).astype(np.float32)
    eb2t = np.tile(eb2t, (3, 1))
    s1 = np.exp(np.minimum(ls1, math.log(100.0))).reshape(NH)
    s2 = np.exp(np.minimum(ls2, math.log(100.0))).reshape(NH)
    scl = np.tile(np.concatenate([s2, s1]).astype(np.float32), (128, 1))
    return eb1, eb2t, np.ascontiguousarray(scl)


def kernel(**inputs):
    kwargs = inputs
    import ml_dtypes
    from concourse.bass_utils import run_bass_kernel_spmd

    bf = ml_dtypes.bfloat16
    qkv = np.asarray(inputs["qkv"], dtype=np.float32)
    anchor = np.asarray(inputs["anchor"], dtype=np.float32)
    table = np.asarray(inputs["table"], dtype=np.float32)
    i_a2w = np.asarray(inputs["index_a2w"]).astype(np.int64)
    i_w2a = np.asarray(inputs["index_w2a"]).astype(np.int64)
    eb1, eb2t, scl = _host_consts(
        table, i_a2w, i_w2a,
        np.asarray(inputs["logit_scale1"], np.float32),
        np.asarray(inputs["logit_scale2"], np.float32),
        np.asarray(inputs["cpb1_w1"], np.float32),
        np.asarray(inputs["cpb1_b1"], np.float32),
        np.asarray(inputs["cpb1_w2"], np.float32),
        np.asarray(inputs["cpb2_w1"], np.float32),
        np.asarray(inputs["cpb2_b1"], np.float32),
        np.asarray(inputs["cpb2_w2"], np.float32),
    )
    eb1 = eb1.astype(bf)
    eb2t = eb2t.astype(bf)
    scl = np.ascontiguousarray(scl.astype(bf))

    in_maps = []
    for c in range(NCORES):
        b = c // 4
        rb = c % 4
        qkv_sh = qkv[b, rb * TOK:(rb + 1) * TOK].reshape(NWR, WS, NWC, WS, C)
        anc_sh = anchor[b, rb * 32:(rb + 1) * 32].reshape(NWR, AWS, NWC, AWS, CO)
        in_maps.append({
            "qkv": np.ascontiguousarray(qkv_sh.astype(bf)),
            "anc": np.ascontiguousarray(anc_sh.astype(bf)),
            "eb1": eb1, "eb2t": eb2t, "scl": scl,
        })

    nc = _get_nc()
    trace = bool(kwargs.get("_trace"))
    tkw = {}
    if trace:
        tkw = dict(trace=True, tmpdir=kwargs.get("_tmpdir"))
    res = run_bass_kernel_spmd(nc, in_maps, list(range(NCORES)), **tkw)
    results = res.results if hasattr(res, "results") else res
    if trace:
        kernel._last_profile = res

    out = np.empty((B, H * WID, CO), dtype=np.float32)
    for c in range(NCORES):
        b = c // 4
        rb = c % 4
        out[b, rb * TOK:(rb + 1) * TOK] = np.asarray(
            results[c]["out"]).astype(np.float32).reshape(TOK, CO)
    return out
